# revision 6
# baseline (speedup 1.0000x reference)
"""Causal self-attention on 8 trn2 NeuronCores.

Sharding: core = (batch b, head-group hg): data-parallel over B=4, tensor-
parallel over head halves (8 heads each). Each core computes, fully on-chip:
  qkv slice GEMM -> flash attention (transposed scores) -> partial out_proj.
Host sums the two head-group partials per batch.

Layout notes (per core):
  xT      [C=1024, T=2048]  x[b] transposed (host)          - f32r
  wqkT    [C, 1024]         q|k weight rows transposed       - f32r
  wvT     [C, 512]          v weight rows transposed         - f32r
  w_outT  [512, 1024]       w_out column-slice transposed    - f32r
  qT/kT   [128, 4, 2048]    (head-pair chunks) x T           - f32r sbuf
  v_ones  [128, 16, 8, 65]  v blocks [tk,64] + ones column   - f32r sbuf
  scores  sT[tk, tq] so softmax denominator comes from the AV matmul's
          ones column; causal masking via additive -1e9 triangle on the
          diagonal 128-blocks, with column-trimmed QK/exp/AV.
"""

import sys

sys.path.insert(0, "/opt/trn_rl_repo")

import numpy as np

import bass_rust

import concourse.bass as bass
import concourse.mybir as mybir
import concourse.tile as tile
from concourse.bass_utils import run_bass_kernel_spmd

F32 = mybir.dt.float32
F32R = mybir.dt.float32r
EXP = mybir.ActivationFunctionType.Exp

B, T, C = 4, 2048, 1024
H, DH = 16, 64
HL = 8  # heads per core
CL = 512  # local c width (HL * DH)
NKB = T // 128  # 16 score row blocks
NQB = T // 512  # 4 score col blocks
NCC = C // 128  # 8 contraction chunks


def _fresh_nop(nc, engine):
    bi = nc.engines[engine].nop(hint="waitsplit")
    cur = nc.cur_bb.bb
    assert cur.instructions and cur.instructions[-1] is bi.ins
    cur.instructions.pop()
    return bi.ins


def legalize_waits(nc, cap=1):
    """walrus accepts very few sync waits per instruction (1 for self-loading
    f32r matmuls; the tail drain would get 10+). Hoist excess on_wait entries
    onto same-engine NoOps inserted right before the oversubscribed
    instruction."""
    n = 0
    for f in nc.m.functions:
        for bb in f.blocks:
            out = []
            dirty = False
            for inst in bb.instructions:
                si = inst.sync_info
                waits = list(si.on_wait) if (si is not None and si.on_wait) else []
                if len(waits) > cap:
                    keep = waits[len(waits) - cap :]
                    for w in waits[: len(waits) - cap]:
                        nop = _fresh_nop(nc, inst.engine)
                        nop.sync_info = bass_rust.SyncInfo(on_wait=[w], on_update=[])
                        out.append(nop)
                        n += 1
                    inst.sync_info = bass_rust.SyncInfo(
                        on_wait=keep,
                        on_update=list(si.on_update) if si.on_update else [],
                    )
                    dirty = True
                out.append(inst)
            if dirty:
                bb.instructions = out
    return n


def build_nc():
    nc = bass.Bass()

    XT = nc.dram_tensor("xt", [C, T], F32R, kind="ExternalInput")
    WQK = nc.dram_tensor("wqk", [C, 1024], F32R, kind="ExternalInput")
    WV = nc.dram_tensor("wv", [C, CL], F32R, kind="ExternalInput")
    WO = nc.dram_tensor("wo", [CL, C], F32R, kind="ExternalInput")
    TRI = nc.dram_tensor("tri", [128, 128], F32, kind="ExternalInput")
    ONES = nc.dram_tensor("ones", [128, 128], F32R, kind="ExternalInput")
    OUT = nc.dram_tensor("out", [T, C], F32, kind="ExternalOutput")

    with tile.TileContext(nc) as tc:
        with (
            tc.tile_pool(name="persist", bufs=1) as persist,
            tc.tile_pool(name="small", bufs=1) as small,
        ):
            qT = persist.tile([128, 4, T], F32R, tag="qT")
            kT = persist.tile([128, 4, T], F32R, tag="kT")
            vo = persist.tile([128, NKB, HL, 65], F32R, tag="vo")
            tri = small.tile([128, 128], F32, tag="tri")
            nc.sync.dma_start(out=tri, in_=TRI[:, :])
            ones = small.tile([128, 128], F32R, tag="ones")
            nc.sync.dma_start(out=ones, in_=ONES[:, :])
            # ones columns of v_ones, set once (f32r memset unsupported -> DMA)
            nc.sync.dma_start(
                out=vo[:, :, :, 64:65],
                in_=ONES.rearrange("p (a b u) -> p a b u", a=NKB, u=1),
            )

            # ---------------- phase 1: qkv ----------------
            with (
                tc.tile_pool(name="p1sb", bufs=1) as p1sb,
                tc.tile_pool(name="p1x", bufs=2) as p1x,
                tc.tile_pool(name="p1ps", bufs=4, space="PSUM") as p1ps,
            ):
                wqk = p1sb.tile([128, NCC, 1024], F32R, tag="wqk")
                nc.sync.dma_start(
                    out=wqk,
                    in_=WQK.rearrange("(cc p) f -> p cc f", p=128),
                )
                wv = p1sb.tile([128, NCC, CL], F32R, tag="wv")
                nc.sync.dma_start(
                    out=wv,
                    in_=WV.rearrange("(cc p) f -> p cc f", p=128),
                )
                for t5 in range(NQB):  # 512-token stripes
                    xt = p1x.tile([128, NCC, 512], F32R, tag="xt")
                    nc.sync.dma_start(
                        out=xt,
                        in_=XT.rearrange("(cc p) t -> p cc t", p=128)[
                            :, :, t5 * 512 : (t5 + 1) * 512
                        ],
                    )
                    for dc in range(8):  # 4 q chunks then 4 k chunks
                        ps = p1ps.tile([128, 512], F32, tag="mm")
                        for ci in range(NCC):
                            nc.tensor.matmul(
                                out=ps,
                                lhsT=wqk[:, ci, dc * 128 : dc * 128 + 128],
                                rhs=xt[:, ci, :],
                                start=(ci == 0),
                                stop=(ci == NCC - 1),
                            )
                        dst = qT if dc < 4 else kT
                        nc.vector.tensor_copy(
                            out=dst[:, dc % 4, t5 * 512 : (t5 + 1) * 512], in_=ps
                        )
                    for tb in range(4):  # v natural blocks
                        ps = p1ps.tile([128, 512], F32, tag="mm")
                        for ci in range(NCC):
                            nc.tensor.matmul(
                                out=ps,
                                lhsT=xt[:, ci, tb * 128 : tb * 128 + 128],
                                rhs=wv[:, ci, :],
                                start=(ci == 0),
                                stop=(ci == NCC - 1),
                            )
                        kb = t5 * 4 + tb
                        nc.scalar.copy(
                            out=vo[:, kb, :, 0:64],
                            in_=ps.rearrange("p (h d) -> p h d", h=HL),
                        )

            # ---------------- phases 2+3 ----------------
            with tc.tile_pool(name="p23", bufs=1) as p23:
                yT = p23.tile([128, 4, T], F32R, tag="yT")
                attention_and_out(nc, tc, qT, kT, vo, yT, tri, ones, WO, OUT)

    legalize_waits(nc)
    return nc


def attention_and_out(nc, tc, qT, kT, vo, yT, tri, ones, WO, OUT):
    if True:
            # ---------------- phase 2: attention ----------------
            with (
                tc.tile_pool(name="p2p", bufs=4) as p2p,
                tc.tile_pool(name="p2n", bufs=2) as p2n,
                tc.tile_pool(name="p2s", bufs=3, space="PSUM") as p2s,
                tc.tile_pool(name="p2y", bufs=2, space="PSUM") as p2y,
                tc.tile_pool(name="p2r", bufs=2, space="PSUM") as p2r,
            ):
                for h in range(HL):
                    hp, hi = h // 2, h % 2
                    pr = slice(64 * hi, 64 * hi + 64)
                    for qb in range(NQB):
                        nkb = 4 * qb + 4
                        y_ps = p2y.tile([128, 512], F32, tag="y")
                        for kb in range(nkb):
                            off = max(0, kb * 128 - qb * 512)
                            s_ps = p2s.tile([128, 512], F32, tag="s")
                            nc.tensor.matmul(
                                out=s_ps[:, off:512],
                                lhsT=kT[pr, hp, kb * 128 : kb * 128 + 128],
                                rhs=qT[pr, hp, qb * 512 + off : qb * 512 + 512],
                                start=True,
                                stop=True,
                            )
                            if off or kb * 128 >= qb * 512:  # diagonal block
                                nc.vector.tensor_add(
                                    out=s_ps[:, off : off + 128],
                                    in0=s_ps[:, off : off + 128],
                                    in1=tri,
                                )
                            p_sb = p2p.tile([128, 512], F32R, tag="p")
                            nc.scalar.activation(
                                out=p_sb[:, off:512],
                                in_=s_ps[:, off:512],
                                func=EXP,
                                scale=0.125,
                            )
                            nc.tensor.matmul(
                                out=y_ps[0:65, off:512],
                                lhsT=vo[:, kb, h, :],
                                rhs=p_sb[:, off:512],
                                start=(kb == 0),
                                stop=(kb == nkb - 1),
                            )
                        # normalization: denom row -> reciprocal -> K=1
                        # broadcast matmul -> scale y rows
                        den = p2n.tile([1, 512], F32, tag="den")
                        nc.scalar.copy(out=den, in_=y_ps[64:65, :])
                        rec = p2n.tile([1, 512], F32R, tag="rec")
                        with nc.allow_low_precision(reason="softmax denom"):
                            nc.vector.reciprocal(out=rec, in_=den)
                        rb_ps = p2r.tile([64, 512], F32, tag="rb")
                        nc.tensor.matmul(
                            out=rb_ps,
                            lhsT=ones[0:1, 0:64],
                            rhs=rec,
                            start=True,
                            stop=True,
                        )
                        rb = p2n.tile([64, 512], F32R, tag="rbs")
                        nc.scalar.copy(out=rb, in_=rb_ps)
                        qs = slice(qb * 512, qb * 512 + 512)
                        if hi == 0:
                            nc.vector.tensor_mul(
                                out=yT[0:64, hp, qs], in0=y_ps[0:64, :], in1=rb
                            )
                        else:
                            stg = p2n.tile([64, 512], F32R, tag="stg")
                            nc.vector.tensor_mul(
                                out=stg, in0=y_ps[0:64, :], in1=rb
                            )
                            nc.sync.dma_start(out=yT[64:128, hp, qs], in_=stg)

            # ---------------- phase 3: out projection ----------------
            with (
                tc.tile_pool(name="p3w", bufs=1) as p3w,
                tc.tile_pool(name="p3o", bufs=3) as p3o,
                tc.tile_pool(name="p3ps", bufs=4, space="PSUM") as p3ps,
            ):
                wo = p3w.tile([128, 4, C], F32R, tag="wo")
                nc.sync.dma_start(
                    out=wo, in_=WO.rearrange("(cc p) f -> p cc f", p=128)
                )
                for tb in range(NKB):
                    o_sb = p3o.tile([128, C], F32, tag="osb")
                    for fh in range(2):
                        o_ps = p3ps.tile([128, 512], F32, tag="o")
                        for ci in range(4):
                            nc.tensor.matmul(
                                out=o_ps,
                                lhsT=yT[:, ci, tb * 128 : tb * 128 + 128],
                                rhs=wo[:, ci, fh * 512 : fh * 512 + 512],
                                start=(ci == 0),
                                stop=(ci == 3),
                            )
                        if fh == 0:
                            nc.vector.tensor_copy(
                                out=o_sb[:, fh * 512 : fh * 512 + 512], in_=o_ps
                            )
                        else:
                            nc.scalar.copy(
                                out=o_sb[:, fh * 512 : fh * 512 + 512], in_=o_ps
                            )
                    nc.sync.dma_start(
                        out=OUT[tb * 128 : tb * 128 + 128, :], in_=o_sb
                    )


_NC_CACHE = None


def _get_nc():
    global _NC_CACHE
    if _NC_CACHE is None:
        _NC_CACHE = build_nc()
    return _NC_CACHE


def prep_in_maps(x, w_qkv, w_out):
    x = np.asarray(x, dtype=np.float32)
    w_qkv = np.asarray(w_qkv, dtype=np.float32)
    w_out = np.asarray(w_out, dtype=np.float32)

    tri = np.where(
        np.arange(128)[:, None] > np.arange(128)[None, :], -1e9, 0.0
    ).astype(np.float32)
    ones_row = np.ones((128, 128), np.float32)

    in_maps = []
    for core in range(8):
        b, hg = core // 2, core % 2
        rows = slice(CL * hg, CL * hg + CL)
        xT = np.ascontiguousarray(x[b].T)
        wqkT = np.ascontiguousarray(
            np.concatenate([w_qkv[0:C][rows], w_qkv[C : 2 * C][rows]], axis=0).T
        )
        wvT = np.ascontiguousarray(w_qkv[2 * C : 3 * C][rows].T)
        woT = np.ascontiguousarray(w_out[:, rows].T)
        in_maps.append(
            {
                "xt": xT,
                "wqk": wqkT,
                "wv": wvT,
                "wo": woT,
                "tri": tri,
                "ones": ones_row,
            }
        )
    return in_maps


def kernel(x, w_qkv, w_out):
    in_maps = prep_in_maps(x, w_qkv, w_out)
    res = run_bass_kernel_spmd(_get_nc(), in_maps, core_ids=list(range(8)))
    outs = [r["out"] for r in res.results]
    y = np.empty((B, T, C), np.float32)
    for b in range(B):
        y[b] = outs[2 * b] + outs[2 * b + 1]
    return y


# revision 13
# speedup vs baseline: 1.1711x; 1.1711x over previous
"""Causal self-attention on 8 trn2 NeuronCores.

Sharding: core = (batch b, head-group hg): data-parallel over B=4, tensor-
parallel over head halves (8 heads each). Each core computes, fully on-chip:
  qkv slice GEMM -> flash attention (transposed scores) -> partial out_proj.
Host sums the two head-group partials per batch.

Layout notes (per core):
  xT      [C=1024, T=2048]  x[b] transposed (host)          - f32r
  wqkT    [C, 1024]         q|k weight rows transposed       - f32r
  wvT     [C, 512]          v weight rows transposed         - f32r
  w_outT  [512, 1024]       w_out column-slice transposed    - f32r
  qT/kT   [128, 4, 2048]    (head-pair chunks) x T           - f32r sbuf
  v_ones  [128, 16, 8, 65]  v blocks [tk,64] + ones column   - f32r sbuf
  scores  sT[tk, tq] so softmax denominator comes from the AV matmul's
          ones column; causal masking via additive -1e9 triangle on the
          diagonal 128-blocks, with column-trimmed QK/exp/AV.
"""

import sys

sys.path.insert(0, "/opt/trn_rl_repo")

import numpy as np

import bass_rust

import concourse.bass as bass
import concourse.mybir as mybir
import concourse.tile as tile
from concourse.bass_utils import run_bass_kernel_spmd

F32 = mybir.dt.float32
F32R = mybir.dt.float32r
EXP = mybir.ActivationFunctionType.Exp
LOG = mybir.ActivationFunctionType.Ln if hasattr(mybir.ActivationFunctionType, 'Ln') else mybir.ActivationFunctionType.Log

B, T, C = 4, 2048, 1024
H, DH = 16, 64
HL = 8  # heads per core
CL = 512  # local c width (HL * DH)
NKB = T // 128  # 16 score row blocks
NQB = T // 512  # 4 score col blocks
NCC = C // 128  # 8 contraction chunks


def _fresh_nop(nc, engine):
    bi = nc.engines[engine].nop(hint="waitsplit")
    cur = nc.cur_bb.bb
    assert cur.instructions and cur.instructions[-1] is bi.ins
    cur.instructions.pop()
    return bi.ins


def legalize_waits(nc, cap=1):
    """walrus accepts very few sync waits per instruction (1 for self-loading
    f32r matmuls; the tail drain would get 10+). Hoist excess on_wait entries
    onto same-engine NoOps inserted right before the oversubscribed
    instruction."""
    n = 0
    for f in nc.m.functions:
        for bb in f.blocks:
            out = []
            dirty = False
            for inst in bb.instructions:
                si = inst.sync_info
                waits = list(si.on_wait) if (si is not None and si.on_wait) else []
                if len(waits) > cap:
                    keep = waits[len(waits) - cap :]
                    for w in waits[: len(waits) - cap]:
                        nop = _fresh_nop(nc, inst.engine)
                        nop.sync_info = bass_rust.SyncInfo(on_wait=[w], on_update=[])
                        out.append(nop)
                        n += 1
                    inst.sync_info = bass_rust.SyncInfo(
                        on_wait=keep,
                        on_update=list(si.on_update) if si.on_update else [],
                    )
                    dirty = True
                out.append(inst)
            if dirty:
                bb.instructions = out
    return n


def build_nc():
    nc = bass.Bass()

    XT = nc.dram_tensor("xt", [C, T], F32R, kind="ExternalInput")
    WQK = nc.dram_tensor("wqk", [C, 1024], F32R, kind="ExternalInput")
    WV = nc.dram_tensor("wv", [C, CL], F32R, kind="ExternalInput")
    WO = nc.dram_tensor("wo", [CL, C], F32R, kind="ExternalInput")
    TRI = nc.dram_tensor("tri", [128, 128], F32, kind="ExternalInput")
    ONES = nc.dram_tensor("ones", [128, 128], F32R, kind="ExternalInput")
    OUT = nc.dram_tensor("out", [T, C], F32, kind="ExternalOutput")
    SCR = nc.dram_tensor("scratch", [32, 512], F32)

    with tile.TileContext(nc) as tc:
        with (
            tc.tile_pool(name="persist", bufs=1) as persist,
            tc.tile_pool(name="small", bufs=1) as small,
        ):
            qT = persist.tile([128, 4, T], F32R, tag="qT")
            kT = persist.tile([128, 4, T], F32R, tag="kT")
            vo = persist.tile([128, NKB, HL, 65], F32R, tag="vo")
            tri = small.tile([128, 128], F32, tag="tri")
            nc.sync.dma_start(out=tri, in_=TRI[:, :])
            ones = small.tile([128, 128], F32R, tag="ones")
            nc.sync.dma_start(out=ones, in_=ONES[:, :])
            # ones columns of v_ones, set once (f32r memset unsupported -> DMA)
            nc.sync.dma_start(
                out=vo[:, :, :, 64:65],
                in_=ONES.rearrange("p (a b u) -> p a b u", a=NKB, u=1),
            )

            # ---------------- phase 1: qkv ----------------
            with (
                tc.tile_pool(name="p1sb", bufs=1) as p1sb,
                tc.tile_pool(name="p1x", bufs=2) as p1x,
                tc.tile_pool(name="p1ps", bufs=4, space="PSUM") as p1ps,
            ):
                wqk = p1sb.tile([128, NCC, 1024], F32R, tag="wqk")
                nc.sync.dma_start(
                    out=wqk,
                    in_=WQK.rearrange("(cc p) f -> p cc f", p=128),
                )
                wv = p1sb.tile([128, NCC, CL], F32R, tag="wv")
                nc.sync.dma_start(
                    out=wv,
                    in_=WV.rearrange("(cc p) f -> p cc f", p=128),
                )
                for t5 in range(NQB):  # 512-token stripes
                    xt = p1x.tile([128, NCC, 512], F32R, tag="xt")
                    nc.sync.dma_start(
                        out=xt,
                        in_=XT.rearrange("(cc p) t -> p cc t", p=128)[
                            :, :, t5 * 512 : (t5 + 1) * 512
                        ],
                    )
                    for dc in range(8):  # 4 q chunks then 4 k chunks
                        ps = p1ps.tile([128, 512], F32, tag="mm")
                        for ci in range(NCC):
                            nc.tensor.matmul(
                                out=ps,
                                lhsT=wqk[:, ci, dc * 128 : dc * 128 + 128],
                                rhs=xt[:, ci, :],
                                start=(ci == 0),
                                stop=(ci == NCC - 1),
                            )
                        dst = qT if dc < 4 else kT
                        nc.vector.tensor_copy(
                            out=dst[:, dc % 4, t5 * 512 : (t5 + 1) * 512], in_=ps
                        )
                    for tb in range(4):  # v natural blocks
                        ps = p1ps.tile([128, 512], F32, tag="mm")
                        for ci in range(NCC):
                            nc.tensor.matmul(
                                out=ps,
                                lhsT=xt[:, ci, tb * 128 : tb * 128 + 128],
                                rhs=wv[:, ci, :],
                                start=(ci == 0),
                                stop=(ci == NCC - 1),
                            )
                        kb = t5 * 4 + tb
                        nc.scalar.copy(
                            out=vo[:, kb, :, 0:64],
                            in_=ps.rearrange("p (h d) -> p h d", h=HL),
                        )

            # ---------------- phases 2+3 ----------------
            with tc.tile_pool(name="p23", bufs=1) as p23:
                yT = p23.tile([128, 4, T], F32R, tag="yT")
                attention_and_out(nc, tc, qT, kT, vo, yT, tri, ones, WO, OUT, SCR)

    legalize_waits(nc)
    return nc


def attention_and_out(nc, tc, qT, kT, vo, yT, tri, ones, WO, OUT, SCR):
    if True:
            # ---------------- phase 2: attention ----------------
            # Per (head-pair, qb) group: all QK matmuls (64x128 tile mode,
            # heads interleaved for T0/T8 row-tile concurrency) with exp'd
            # p blocks staged in SBUF, then the two AV batches (128x128
            # mode), then normalization whose broadcast matmul is K=64 so it
            # shares the QK tile mode: 2 PE mode switches per group instead
            # of 2 per matmul.
            with (
                tc.tile_pool(name="p2p", bufs=33) as p2p,
                tc.tile_pool(name="p2n", bufs=2) as p2n,
                tc.tile_pool(name="p2s", bufs=4, space="PSUM") as p2s,
                tc.tile_pool(name="p2y", bufs=3, space="PSUM") as p2y,
            ):
                for hp in range(HL // 2):
                    prs = [slice(0, 64), slice(64, 128)]
                    for qb in range(NQB):
                        nkb = 4 * qb + 4
                        qs = slice(qb * 512, qb * 512 + 512)
                        y_ps = [
                            p2y.tile([65, 512], F32, tag="y", name=f"y{i}")
                            for i in range(2)
                        ]
                        p_sb = [[None] * nkb, [None] * nkb]
                        # QK + exp for both heads, interleaved
                        for kb in range(nkb):
                            off = max(0, kb * 128 - qb * 512)
                            s_ps = [None, None]
                            for hi in range(2):
                                s_ps[hi] = p2s.tile(
                                    [128, 512], F32, tag="s", name=f"s{hi}"
                                )
                                nc.tensor.matmul(
                                    out=s_ps[hi][:, off:512],
                                    lhsT=kT[prs[hi], hp, kb * 128 : kb * 128 + 128],
                                    rhs=qT[prs[hi], hp, qb * 512 + off : qb * 512 + 512],
                                    start=True,
                                    stop=True,
                                )
                            diag = kb * 128 >= qb * 512
                            for hi in range(2):
                                if diag:
                                    nc.vector.tensor_add(
                                        out=s_ps[hi][:, off : off + 128],
                                        in0=s_ps[hi][:, off : off + 128],
                                        in1=tri,
                                    )
                                pt = p2p.tile(
                                    [128, 512], F32R, tag="p", name=f"p{hi}"
                                )
                                nc.scalar.activation(
                                    out=pt[:, off:512],
                                    in_=s_ps[hi][:, off:512],
                                    func=EXP,
                                    scale=0.125,
                                )
                                p_sb[hi][kb] = pt
                        # AV batches (one per head)
                        for hi in range(2):
                            h = 2 * hp + hi
                            for kb in range(nkb):
                                off = max(0, kb * 128 - qb * 512)
                                nc.tensor.matmul(
                                    out=y_ps[hi][:, off:512],
                                    lhsT=vo[:, kb, h, :],
                                    rhs=p_sb[hi][kb][:, off:512],
                                    start=(kb == 0),
                                    stop=(kb == nkb - 1),
                                )
                        # normalization (K=64 broadcast matmul, QK tile mode)
                        for hi in range(2):
                            # 1/denom = exp(-ln(denom)) on ACT (row 64), then
                            # broadcast across partitions via a DRAM bounce
                            lnd = p2n.tile([128, 512], F32, tag="lnd")
                            nc.scalar.activation(
                                out=lnd[64:65, :],
                                in_=y_ps[hi][64:65, :],
                                func=LOG,
                            )
                            nc.scalar.activation(
                                out=lnd[64:65, :],
                                in_=lnd[64:65, :],
                                func=EXP,
                                scale=-1.0,
                            )
                            g = (hp * NQB + qb) * 2 + hi
                            nc.sync.dma_start(out=SCR[g, :], in_=lnd[64:65, :])
                            rb = p2n.tile([64, 512], F32, tag="rbs")
                            nc.sync.dma_start(
                                out=rb,
                                in_=SCR[g : g + 1, :].broadcast_to((64, 512)),
                            )
                            stg = p2n.tile([64, 512], F32R, tag="stg")
                            nc.vector.tensor_mul(
                                out=stg,
                                in0=y_ps[hi][0:64, :],
                                in1=rb,
                            )
                            nc.sync.dma_start(
                                out=yT[64 * hi : 64 * hi + 64, hp, qs],
                                in_=stg,
                            )

            # ---------------- phase 3: out projection ----------------
            with (
                tc.tile_pool(name="p3w", bufs=1) as p3w,
                tc.tile_pool(name="p3o", bufs=3) as p3o,
                tc.tile_pool(name="p3ps", bufs=4, space="PSUM") as p3ps,
            ):
                wo = p3w.tile([128, 4, C], F32R, tag="wo")
                nc.sync.dma_start(
                    out=wo, in_=WO.rearrange("(cc p) f -> p cc f", p=128)
                )
                for tb in range(NKB):
                    o_sb = p3o.tile([128, C], F32, tag="osb")
                    for fh in range(2):
                        o_ps = p3ps.tile([128, 512], F32, tag="o")
                        for ci in range(4):
                            nc.tensor.matmul(
                                out=o_ps,
                                lhsT=yT[:, ci, tb * 128 : tb * 128 + 128],
                                rhs=wo[:, ci, fh * 512 : fh * 512 + 512],
                                start=(ci == 0),
                                stop=(ci == 3),
                            )
                        if fh == 0:
                            nc.vector.tensor_copy(
                                out=o_sb[:, fh * 512 : fh * 512 + 512], in_=o_ps
                            )
                        else:
                            nc.scalar.copy(
                                out=o_sb[:, fh * 512 : fh * 512 + 512], in_=o_ps
                            )
                    nc.sync.dma_start(
                        out=OUT[tb * 128 : tb * 128 + 128, :], in_=o_sb
                    )


_NC_CACHE = None


def _get_nc():
    global _NC_CACHE
    if _NC_CACHE is None:
        _NC_CACHE = build_nc()
    return _NC_CACHE


def prep_in_maps(x, w_qkv, w_out):
    x = np.asarray(x, dtype=np.float32)
    w_qkv = np.asarray(w_qkv, dtype=np.float32)
    w_out = np.asarray(w_out, dtype=np.float32)

    tri = np.where(
        np.arange(128)[:, None] > np.arange(128)[None, :], -1e9, 0.0
    ).astype(np.float32)
    ones_row = np.ones((128, 128), np.float32)

    in_maps = []
    for core in range(8):
        b, hg = core // 2, core % 2
        rows = slice(CL * hg, CL * hg + CL)
        xT = np.ascontiguousarray(x[b].T)
        wqkT = np.ascontiguousarray(
            np.concatenate([w_qkv[0:C][rows], w_qkv[C : 2 * C][rows]], axis=0).T
        )
        wvT = np.ascontiguousarray(w_qkv[2 * C : 3 * C][rows].T)
        woT = np.ascontiguousarray(w_out[:, rows].T)
        in_maps.append(
            {
                "xt": xT,
                "wqk": wqkT,
                "wv": wvT,
                "wo": woT,
                "tri": tri,
                "ones": ones_row,
            }
        )
    return in_maps


def kernel(x, w_qkv, w_out):
    in_maps = prep_in_maps(x, w_qkv, w_out)
    res = run_bass_kernel_spmd(_get_nc(), in_maps, core_ids=list(range(8)))
    outs = [r["out"] for r in res.results]
    y = np.empty((B, T, C), np.float32)
    for b in range(B):
        y[b] = outs[2 * b] + outs[2 * b + 1]
    return y
